# revision 1
# baseline (speedup 1.0000x reference)
"""Trainium2 Bass kernel for nn_DINOBevAligner (BEVFormer-style view aligner).

Strategy (8 NeuronCores, query-sector sharded, zero cross-core comm),
flipped "query-on-partitions" dataflow:
  - 2500 BEV queries az-sorted into 8 sectors of 320 (384-padded = 3 query
    blocks of 128 per core).  Each core ships only the image columns its
    sector samples (x-major, view-aligned 128-token tiles, bf16).
  - Gather matmul per (token-tile t, query-block b) pair with banded
    structure: psum[q(128 part), ch(768)] += W'[t,b].T @ tok[t] where
    W' = bilinear-weights * softplus(w_view) (host) * rsqrt(var_t+eps)
    (device, from bn_stats per token).
  - Per-query epilogue is 128-way parallel: bn_stats on the PSUM acc gives
    the fused-feature mean K(q) and variance directly;
    A = rsqrt(var + eps*den'^2); y = A*(r - K*s1) + g2 with
    r = grouped (vals * acc) reduce via two stride-3 adds.
  - Host precomputes the small parameter vectors (softmax(logits)*gamma
    rows, den from cnt@softplus(w_view)) exactly as it precomputes the
    projection weight blocks from lidar2img.
  - Per-block epilogues + output DMAs overlap later blocks' matmuls.
  - PE clock-gate (HAM) warmed with dummy matmuls during the DMA prologue.
"""
import sys

sys.path.insert(0, "/opt/trn_rl_repo")

import numpy as np
import ml_dtypes

BEV_H, BEV_W = 50, 50
D_PILLAR = 4
PC = (-51.2, -51.2, -5.0, 51.2, 51.2, 3.0)
S_IMG = 518.0
LN_EPS = 1e-5
FUSE_EPS = 1e-6
C_CTX = 256
Q = BEV_H * BEV_W
NCORE = 8
SEC = 320                    # queries per core (2560 padded)
QB = 128                     # query block (partition dim)
NB = 3                       # blocks per core
TOK_TILE = 128
V = 6
C = 768
TW = 776                     # tok slot SBUF width (768 used + pad)
N_WARM = 46                  # PE clock-gate warmup matmuls


# ----------------------------------------------------------------- host math
def _projection_np(lidar2img):
    dt = np.float32
    Z = int(round(PC[5] - PC[2]))
    zs = (np.linspace(0.5, Z - 0.5, D_PILLAR, dtype=dt) / dt(Z))[:, None, None]
    xs = (np.linspace(0.5, BEV_W - 0.5, BEV_W, dtype=dt) / dt(BEV_W))[None, None, :]
    ys = (np.linspace(0.5, BEV_H - 0.5, BEV_H, dtype=dt) / dt(BEV_H))[None, :, None]
    x, y, z = np.broadcast_arrays(xs, ys, zs)
    ref = np.stack([x, y, z], axis=-1).reshape(D_PILLAR, Q, 3).astype(dt)
    ref = ref * np.array([PC[3] - PC[0], PC[4] - PC[1], PC[5] - PC[2]], dt) \
        + np.array([PC[0], PC[1], PC[2]], dt)
    ref4 = np.concatenate([ref, np.ones_like(ref[..., :1])], axis=-1)
    pts = np.einsum('bvij,dqj->bdvqi', lidar2img.astype(dt), ref4)
    zc = pts[..., 2]
    valid = zc > 1e-5
    uv = pts[..., :2] / np.maximum(zc, dt(1e-5))[..., None] / dt(S_IMG)
    u, v = uv[..., 0], uv[..., 1]
    valid = valid & (u > 0.0) & (u < 1.0) & (v > 0.0) & (v < 1.0)
    tr = lambda a: np.transpose(a, (0, 2, 3, 1))
    return tr(u), tr(v), tr(valid)


def build_plan(lidar2img, patch_h, patch_w):
    dt = np.float32
    Hp, Wp = int(patch_h), int(patch_w)
    u, v, valid = _projection_np(lidar2img)
    u, v, valid = u[0], v[0], valid[0]              # (V,Q,D)

    x_p = (u * dt(S_IMG) + dt(0.5)) / dt(S_IMG) * dt(Wp) - dt(0.5)
    y_p = (v * dt(S_IMG) + dt(0.5)) / dt(S_IMG) * dt(Hp) - dt(0.5)
    x0 = np.floor(x_p); fx = x_p - x0; x0 = x0.astype(np.int64)
    y0 = np.floor(y_p); fy = y_p - y0; y0 = y0.astype(np.int64)
    m = valid.astype(dt)
    cnt = m.sum(axis=-1)                            # (V,Q)

    toks = np.full((V, Q, D_PILLAR, 4), -1, dtype=np.int64)
    wts = np.zeros((V, Q, D_PILLAR, 4), dtype=dt)
    ci = 0
    for dx in (0, 1):
        for dy in (0, 1):
            xi, yi = x0 + dx, y0 + dy
            inb = (xi >= 0) & (xi < Wp) & (yi >= 0) & (yi < Hp)
            w = np.where(dx, fx, 1 - fx) * np.where(dy, fy, 1 - fy) * inb.astype(dt)
            w = w * m
            n_xmaj = np.clip(xi, 0, Wp - 1) * Hp + np.clip(yi, 0, Hp - 1)
            live = (w != 0) & inb
            toks[..., ci] = np.where(live, n_xmaj, -1)
            wts[..., ci] = np.where(live, w, 0)
            ci += 1
    tk = toks.reshape(V, Q, 16)
    wt = wts.reshape(V, Q, 16)

    qy, qx = np.divmod(np.arange(Q), BEV_W)
    az = np.arctan2(qy - (BEV_H - 1) / 2.0, qx - (BEV_W - 1) / 2.0)
    perm = np.argsort(az, kind='stable').astype(np.int64)

    cores = []
    for k in range(NCORE):
        qs = perm[k * SEC:min((k + 1) * SEC, Q)]
        # views ordered by first local query using them -> chain-ordered
        # tiles, so per-block tile sets are near-contiguous slot intervals
        vorder = []
        for vv in range(V):
            msk = wt[vv][qs] != 0
            if not msk.any():
                continue
            first_q = int(np.nonzero(msk.any(axis=1))[0].min())
            vorder.append((first_q, vv))
        vorder.sort()
        views = []
        base = 0
        for _, vv in vorder:
            msk = wt[vv][qs] != 0
            cols = tk[vv][qs][msk] // Hp
            xlo, xhi = int(cols.min()), int(cols.max())
            ntok = (xhi - xlo + 1) * Hp
            ntile = (ntok + TOK_TILE - 1) // TOK_TILE
            views.append((vv, xlo, ntok, base))
            base += ntile
        cores.append(dict(qs=qs, views=views, ntil=base))
    NTIL = max(c["ntil"] for c in cores)

    for k, ck in enumerate(cores):
        qs = ck["qs"]; nq = len(qs)
        Wm = np.zeros((NTIL * TOK_TILE, NB * QB), dt)
        tilev = np.zeros(NTIL, np.int64)
        for (vv, xlo, ntok, base) in ck["views"]:
            ntile = (ntok + TOK_TILE - 1) // TOK_TILE
            tilev[base:base + ntile] = vv
            tkv = tk[vv][qs]; wtv = wt[vv][qs]
            rows, cols16 = np.nonzero(wtv)
            n = tkv[rows, cols16]
            l = (n // Hp - xlo) * Hp + (n % Hp) + base * TOK_TILE
            np.add.at(Wm, (l, rows), wtv[rows, cols16])
        sets = []
        for b in range(NB):
            wb = Wm[:, b * QB:(b + 1) * QB].reshape(NTIL, TOK_TILE, QB)
            used = [t for t in range(NTIL) if wb[t].any()]
            sets.append((min(used), max(used) + 1) if used else (0, 0))
        cntq = np.zeros((NB * QB, V), dt)
        cntq[:nq] = cnt.T[qs]
        ck["W"] = Wm; ck["sets"] = sets; ck["cntq"] = cntq; ck["tilev"] = tilev

    bands = []
    for b in range(NB):
        lo = min(c["sets"][b][0] for c in cores)
        hi = max(c["sets"][b][1] for c in cores)
        bands.append((lo, hi))
    pairs = [(t, b) for b in range(NB) for t in range(*bands[b])]
    return dict(perm=perm, cores=cores, NTIL=NTIL, Hp=Hp, Wp=Wp,
                bands=bands, pairs=pairs)


# -------------------------------------------------------------- bass program
def build_program(NTIL, bands, pairs):
    import concourse.bass as bass
    import concourse.bacc as bacc
    import concourse.tile as tile
    from concourse import mybir

    f32 = mybir.dt.float32
    bf16 = mybir.dt.bfloat16
    AF = mybir.ActivationFunctionType
    ALU = mybir.AluOpType
    NPAIR = len(pairs)

    nc = bacc.Bacc("TRN2", target_bir_lowering=False, debug=False,
                   num_devices=NCORE)

    tok_d = nc.dram_tensor("tok", [128, NTIL * C], bf16, kind="ExternalInput")
    w_d = nc.dram_tensor("wmat", [128, NPAIR * 128], bf16, kind="ExternalInput")
    # vals row broadcast [0:768], s1 [768:1024], g2 [1024:1280],
    # d2 = LN_EPS*den'^2 per block [1280:1283] (+pad)
    cst_d = nc.dram_tensor("cst", [128, 1288], bf16, kind="ExternalInput")
    out_d = nc.dram_tensor("out", [128, NB * C_CTX], bf16,
                            kind="ExternalOutput")

    tgrp = [(a, min(a + 3, NTIL)) for a in range(0, NTIL, 3)]
    poff = np.cumsum([0] + [hi - lo for lo, hi in bands]).tolist()

    with tile.TileContext(nc) as tc:
        with (
            tc.tile_pool(name="sb", bufs=1) as sb,
            tc.tile_pool(name="psum", bufs=1, space="PSUM") as ps,
        ):
            nc.scalar.add_instruction(mybir.InstLoadActFuncSet(
                name=f"I-{nc.next_id()}", act_func_set_id=6, ins=[], outs=[]))

            # ---------------- tiles
            tokS = sb.tile([128, NTIL, TW], bf16, tag="tokS")
            wS = sb.tile([128, NPAIR, 128], bf16, tag="wS")
            cstS = sb.tile([128, 1288], bf16, tag="cstS")
            zerS = sb.tile([128, 256], bf16, tag="zerS")
            bnS = sb.tile([128, NTIL, 2, 6], f32, tag="bnS")
            mvS = sb.tile([128, NTIL, 2], f32, tag="mvS")
            sS = sb.tile([128, NTIL], f32, tag="sS")
            muA = sb.tile([128, NTIL], f32, tag="muA")
            sqA = sb.tile([128, NTIL], f32, tag="sqA")
            vtmp = sb.tile([128, NTIL], f32, tag="vtmp")
            junkA = sb.tile([128, C], bf16, tag="junkA")
            bnA = sb.tile([128, NB, 2, 6], f32, tag="bnA")
            kvS = sb.tile([128, NB, 2], f32, tag="kvS")
            accC = sb.tile([128, NB, C], bf16, tag="accC")
            scrV = sb.tile([128, C], bf16, tag="scrV")
            rS = sb.tile([128, NB, C_CTX], bf16, tag="rS")
            t1S = sb.tile([128, NB, C_CTX], bf16, tag="t1S")
            t2S = sb.tile([128, NB, C_CTX], bf16, tag="t2S")
            t3S = sb.tile([128, NB, C_CTX], bf16, tag="t3S")
            zS = sb.tile([128, NB], f32, tag="zS")
            aS = sb.tile([128, NB], f32, tag="aS")
            yS = sb.tile([128, NB, C_CTX], bf16, tag="yS")

            pb = [ps.tile([128, 2, 512], f32, tag=f"pb{b}", name=f"pb{b}")
                  for b in range(NB)]
            wup = ps.tile([128, 2, 512], f32, tag="pb2")   # alias of pb2

            valsB = cstS[:, 0:768]
            s1B = cstS[:, 768:1024]
            g2B = cstS[:, 1024:1280]
            d2B = cstS[:, 1280:1280 + NB]

            # ---------------- DMA issue
            nc.sync.dma_start(out=cstS[:], in_=cst_d.ap())
            tok_v = tok_d.ap().rearrange("p (t c) -> p t c", c=C)
            w_v = w_d.ap().rearrange("p (n q) -> p n q", q=128)

            def dma_tok(gi, eng):
                a, b = tgrp[gi]
                eng.dma_start(out=tokS[:, a:b, 0:C],
                              in_=tok_v[:, a:b, :])

            def dma_w(b, eng):
                eng.dma_start(out=wS[:, poff[b]:poff[b + 1], :],
                              in_=w_v[:, poff[b]:poff[b + 1], :])

            dma_tok(0, nc.gpsimd)
            dma_tok(1, nc.gpsimd)
            dma_w(0, nc.gpsimd)
            for gi in range(2, len(tgrp)):
                dma_tok(gi, nc.gpsimd)
                if gi == 2:
                    dma_w(1, nc.gpsimd)
            if len(tgrp) <= 2:
                dma_w(1, nc.gpsimd)
            dma_w(2, nc.gpsimd)

            nc.vector.memset(zerS[:], 0.0)

            # ---------------- PE warmups (clock-gate)
            for _ in range(N_WARM):
                nc.tensor.matmul(wup[0:64, 0, 0:256], lhsT=zerS[:, 0:64],
                                 rhs=zerS[:], start=True, stop=True,
                                 skip_group_check=True)

            # ---------------- per-group stats (DVE bn + ACT accum split)
            act_tiles = {2, 4}
            nc.vector.memset(mvS[:], 1.0)

            def act_combine(t):
                # var for an ACT-stat tile from (sum, sumsq)
                nc.vector.tensor_scalar(
                    out=vtmp[:, t:t + 1], in0=muA[:, t:t + 1],
                    scalar1=1.0 / C, scalar2=None, op0=ALU.mult)
                nc.vector.tensor_tensor(out=vtmp[:, t:t + 1],
                                        in0=vtmp[:, t:t + 1],
                                        in1=vtmp[:, t:t + 1], op=ALU.mult)
                nc.vector.tensor_scalar(
                    out=sS[:, t:t + 1], in0=sqA[:, t:t + 1],
                    scalar1=1.0 / C, scalar2=LN_EPS, op0=ALU.mult,
                    op1=ALU.add)
                nc.vector.tensor_tensor(out=sS[:, t:t + 1],
                                        in0=sS[:, t:t + 1],
                                        in1=vtmp[:, t:t + 1],
                                        op=ALU.subtract)
                nc.scalar.activation(out=sS[:, t:t + 1], in_=sS[:, t:t + 1],
                                     func=AF.Ln)
                nc.scalar.activation(out=sS[:, t:t + 1], in_=sS[:, t:t + 1],
                                     func=AF.Exp, scale=-0.5)

            def stats_group(gi):
                a, b = tgrp[gi]
                dve_t = [t for t in range(a, b) if t not in act_tiles]
                for t in dve_t:
                    nc.vector.bn_stats(out=bnS[:, t, 0, :],
                                       in_=tokS[:, t, 0:384])
                    nc.vector.bn_stats(out=bnS[:, t, 1, :],
                                       in_=tokS[:, t, 384:768])
                for t in range(a, b):
                    if t in act_tiles:
                        nc.scalar.activation(out=junkA[:], in_=tokS[:, t, 0:C],
                                             func=AF.Copy,
                                             accum_out=muA[:, t:t + 1])
                        nc.scalar.activation(out=junkA[:], in_=tokS[:, t, 0:C],
                                             func=AF.Square,
                                             accum_out=sqA[:, t:t + 1])
                for t in dve_t:
                    nc.vector.bn_aggr(out=mvS[:, t, :], in_=bnS[:, t, :, :])
                if dve_t:
                    # inv for DVE tiles (contiguous run a..)
                    d0, d1 = dve_t[0], dve_t[-1] + 1
                    nc.vector.tensor_scalar(
                        out=sS[:, d0:d1], in0=mvS[:, d0:d1, 1],
                        scalar1=LN_EPS, scalar2=None, op0=ALU.add)
                    nc.scalar.activation(out=sS[:, d0:d1], in_=sS[:, d0:d1],
                                         func=AF.Ln)
                    nc.scalar.activation(out=sS[:, d0:d1], in_=sS[:, d0:d1],
                                         func=AF.Exp, scale=-0.5)
                for t in range(a, b):
                    if t in act_tiles:
                        act_combine(t)

            def scale_pair_act(p, t):
                nc.scalar.activation(out=wS[:, p, :], in_=wS[:, p, :],
                                     func=AF.Copy, scale=sS[:, t:t + 1])

            def scale_sub(b, ta, tb, eng=None):
                # scale pairs of block b whose tiles fall in [ta, tb)
                lo, hi = bands[b]
                ta, tb = max(ta, lo), min(tb, hi)
                if ta >= tb:
                    return
                p0 = poff[b] + ta - lo
                n = tb - ta
                eng = eng or nc.vector
                with nc.allow_low_precision(reason="bf16 W row scale"):
                    eng.tensor_tensor(
                        out=wS[:, p0:p0 + n, :],
                        in0=wS[:, p0:p0 + n, :],
                        in1=sS[:, ta:tb].unsqueeze(2)
                        .broadcast_to([128, n, 128]),
                        op=ALU.mult)

            def pair_mms(p, t, b):
                lo, hi = bands[b]
                nc.tensor.matmul(pb[b][:, 0, 0:384],
                                 lhsT=wS[:, p, :], rhs=tokS[:, t, 0:384],
                                 start=(t == lo), stop=(t == hi - 1),
                                 skip_group_check=True)
                nc.tensor.matmul(pb[b][:, 1, 0:384],
                                 lhsT=wS[:, p, :], rhs=tokS[:, t, 384:768],
                                 start=(t == lo), stop=(t == hi - 1),
                                 skip_group_check=True)

            # ---------------- per-block epilogue (r, K, var, y, out)
            def epilogue(b, split=False):
                acb = accC[:, b, :]
                if split:
                    nc.scalar.copy(out=acb[:, 0:384], in_=pb[b][:, 0, 0:384])
                    with nc.allow_low_precision(reason="bf16 acc copy"):
                        nc.vector.tensor_copy(out=acb[:, 384:768],
                                              in_=pb[b][:, 1, 0:384])
                    with nc.allow_low_precision(reason="bf16 vals*acc"):
                        nc.gpsimd.tensor_tensor(out=scrV[:, 0:384],
                                                in0=acb[:, 0:384],
                                                in1=valsB[:, 0:384],
                                                op=ALU.mult)
                        nc.vector.tensor_tensor(out=scrV[:, 384:768],
                                                in0=acb[:, 384:768],
                                                in1=valsB[:, 384:768],
                                                op=ALU.mult)
                else:
                    nc.scalar.copy(
                        out=acb.rearrange("p (h c) -> p h c", h=2),
                        in_=pb[b][:, :, 0:384])
                    with nc.allow_low_precision(reason="bf16 vals*acc"):
                        nc.gpsimd.tensor_tensor(out=scrV[:], in0=acb,
                                                in1=valsB, op=ALU.mult)
                v3 = scrV[:].rearrange("p (k g) -> p k g", g=3)
                with nc.allow_low_precision(reason="bf16 r"):
                    nc.vector.tensor_tensor(out=rS[:, b, :], in0=v3[:, :, 0],
                                            in1=v3[:, :, 1], op=ALU.add)
                    nc.vector.tensor_tensor(out=rS[:, b, :], in0=rS[:, b, :],
                                            in1=v3[:, :, 2], op=ALU.add)
                nc.vector.bn_stats(out=bnA[:, b, 0, :], in_=pb[b][:, 0, 0:384])
                nc.vector.bn_stats(out=bnA[:, b, 1, :], in_=pb[b][:, 1, 0:384])
                nc.vector.bn_aggr(out=kvS[:, b, :], in_=bnA[:, b, :, :])
                # A = rsqrt(var + d2)
                nc.vector.tensor_tensor(out=zS[:, b:b + 1],
                                        in0=kvS[:, b, 1:2],
                                        in1=d2B[:, b:b + 1], op=ALU.add)
                nc.scalar.activation(out=zS[:, b:b + 1], in_=zS[:, b:b + 1],
                                     func=AF.Ln)
                nc.scalar.activation(out=aS[:, b:b + 1], in_=zS[:, b:b + 1],
                                     func=AF.Exp, scale=-0.5)
                # y = A*(r - K*s1) + g2
                yeng = nc.vector if b == NB - 1 else nc.gpsimd
                nc.scalar.activation(out=t1S[:, b, :], in_=s1B,
                                     func=AF.Copy, scale=kvS[:, b, 0:1])
                with nc.allow_low_precision(reason="bf16 y chain"):
                    yeng.tensor_tensor(out=t2S[:, b, :], in0=rS[:, b, :],
                                       in1=t1S[:, b, :], op=ALU.subtract)
                nc.scalar.activation(out=t3S[:, b, :], in_=t2S[:, b, :],
                                     func=AF.Copy, scale=aS[:, b:b + 1])
                with nc.allow_low_precision(reason="bf16 y out"):
                    yeng.tensor_tensor(out=yS[:, b, :],
                                       in0=t3S[:, b, :],
                                       in1=g2B, op=ALU.add)
                nc.sync.dma_start(
                    out=out_d.ap().rearrange("p (b k) -> p b k", k=C_CTX)
                    [:, b, :], in_=yS[:, b, :])

            # ---------------- emission flow: tile-major across blocks
            done_epi = set()
            for gi in range(len(tgrp)):
                a, b = tgrp[gi]
                stats_group(gi)
                for bb in range(NB):
                    scale_sub(bb, a, b, eng=nc.gpsimd)
                for t in range(a, b):
                    for bb in range(NB):
                        lo, hi = bands[bb]
                        if lo <= t < hi:
                            pair_mms(poff[bb] + t - lo, t, bb)
                for bb in range(NB):
                    lo, hi = bands[bb]
                    if hi <= b and bb not in done_epi:
                        done_epi.add(bb)
                        epilogue(bb, split=(bb > 0))
            for bb in range(NB):
                if bb not in done_epi:
                    epilogue(bb, split=(bb > 0))

    nc.compile()
    return nc


# ------------------------------------------------------------------- driver
def make_in_maps(inputs, plan):
    lt = np.asarray(inputs["last_tokens"], np.float32)
    gamma = np.asarray(inputs["post_gamma"], np.float32).ravel()
    beta = np.asarray(inputs["post_beta"], np.float32).ravel()
    logits = np.asarray(inputs["logits"], np.float32).reshape(C_CTX, 3)
    w_view = np.asarray(inputs["w_view"], np.float32).ravel()

    NTIL = plan["NTIL"]; Hp, Wp = plan["Hp"], plan["Wp"]
    pairs = plan["pairs"]
    NPAIR = len(pairs)

    # small-parameter preprocessing (same spirit as lidar2img -> W blocks)
    wvp = np.log1p(np.exp(w_view))                       # softplus
    ex = np.exp(logits - logits.max(-1, keepdims=True))
    wg = ex / ex.sum(-1, keepdims=True)                  # softmax (256,3)
    vals = (wg * gamma.reshape(C_CTX, 3)).reshape(-1)    # (768,)
    s1 = vals.reshape(C_CTX, 3).sum(-1)                  # (256,)
    g2 = (wg * beta.reshape(C_CTX, 3)).sum(-1)           # (256,)

    xm = np.transpose(lt[0].reshape(V, Hp, Wp, C),
                      (0, 2, 1, 3)).reshape(V, Wp * Hp, C)

    in_maps = []
    for k in range(NCORE):
        ck = plan["cores"][k]
        arr = np.zeros((NTIL * TOK_TILE, C), np.float32)
        for (vv, xlo, ntok, base) in ck["views"]:
            arr[base * TOK_TILE:base * TOK_TILE + ntok] = \
                xm[vv, xlo * Hp:xlo * Hp + ntok]
        tokp = arr.reshape(NTIL, TOK_TILE, C).transpose(1, 0, 2) \
            .astype(ml_dtypes.bfloat16)
        # W scaled by softplus(w_view) per tile's view on host
        Wm = (ck["W"].reshape(NTIL, TOK_TILE, NB * QB)
              * wvp[ck["tilev"]][:, None, None]) \
            .reshape(NTIL * TOK_TILE, NB * QB).astype(ml_dtypes.bfloat16)
        wp = np.zeros((128, NPAIR, 128), ml_dtypes.bfloat16)
        for p, (t, b) in enumerate(pairs):
            wp[:, p, :] = Wm[t * TOK_TILE:(t + 1) * TOK_TILE,
                             b * QB:(b + 1) * QB]
        den = ck["cntq"] @ wvp + FUSE_EPS                # (384,)
        d2 = (LN_EPS * den * den).reshape(NB, QB).T      # (128, NB)
        cst = np.zeros((128, 1288), np.float32)
        cst[:, 0:768] = vals[None]
        cst[:, 768:1024] = s1[None]
        cst[:, 1024:1280] = g2[None]
        cst[:, 1280:1280 + NB] = d2
        in_maps.append({
            "tok": np.ascontiguousarray(tokp.reshape(128, NTIL * C)),
            "wmat": np.ascontiguousarray(wp.reshape(128, NPAIR * 128)),
            "cst": np.ascontiguousarray(cst.astype(ml_dtypes.bfloat16)),
        })
    return in_maps


def assemble_output(results, plan):
    Y = np.zeros((Q, C_CTX), np.float32)
    perm = plan["perm"]
    for k in range(NCORE):
        arr = np.asarray(results[k]["out"], np.float32) \
            .reshape(128, NB, C_CTX)
        qs = perm[k * SEC:min((k + 1) * SEC, Q)]
        nq = len(qs)
        yk = arr.transpose(1, 0, 2).reshape(NB * QB, C_CTX)
        Y[qs] = yk[:nq]
    return np.ascontiguousarray(
        Y.reshape(1, BEV_H, BEV_W, C_CTX).transpose(0, 3, 1, 2))


_CACHE = {}


def _get_program(lidar2img, patch_h, patch_w):
    key = (lidar2img.tobytes(), int(patch_h), int(patch_w))
    if key not in _CACHE:
        plan = build_plan(lidar2img, patch_h, patch_w)
        nc = build_program(plan["NTIL"], plan["bands"], plan["pairs"])
        _CACHE[key] = (plan, nc)
    return _CACHE[key]


def _install_ntff_shim():
    """Provide antenv.axon_hooks (absent in this image) so trace=True can
    capture NTFF profiles via the axon PJRT .so. Used only by test.py."""
    import types
    import ctypes
    import contextlib
    if "antenv.axon_hooks" in sys.modules:
        return
    so_path = "/opt/axon/libaxon_pjrt.so"
    lib = ctypes.CDLL(so_path)
    if not hasattr(lib, "axon_start_nrt_profile"):
        return
    lib.axon_start_nrt_profile.argtypes = [
        ctypes.POINTER(ctypes.c_int64), ctypes.c_size_t]
    lib.axon_start_nrt_profile.restype = ctypes.c_int64
    lib.axon_stop_nrt_profile.argtypes = [ctypes.c_char_p]
    lib.axon_stop_nrt_profile.restype = ctypes.c_int64

    @contextlib.contextmanager
    def _hook(output_dir, device_ids):
        import jax
        jax.devices()
        if device_ids:
            ids = (ctypes.c_int64 * len(device_ids))(*device_ids)
            rc = lib.axon_start_nrt_profile(ids, len(device_ids))
        else:
            rc = lib.axon_start_nrt_profile(None, 0)
        if rc != 0:
            raise RuntimeError(f"axon_start_nrt_profile rc={rc}")
        try:
            yield
        finally:
            n = lib.axon_stop_nrt_profile(str(output_dir).encode())
            print(f"ntff profile: {n} file(s) -> {output_dir}", file=sys.stderr)

    mod = types.ModuleType("antenv.axon_hooks")
    mod.get_axon_ntff_profile_hook = lambda: _hook
    mod.set_axon_ntff_profile_hook = lambda h: None
    sys.modules["antenv.axon_hooks"] = mod
    import antenv
    antenv.axon_hooks = mod


def kernel(last_tokens, lidar2img, w_view, post_gamma, post_beta, logits,
           patch_h, patch_w, _trace=False):
    import concourse.bass_utils as bu
    from concourse.bass_utils import run_bass_kernel_spmd
    if _trace:
        _install_ntff_shim()
        bu.upload_artifacts = lambda tmpdir: "local://" + str(tmpdir)
    inputs = dict(last_tokens=np.asarray(last_tokens),
                  lidar2img=np.asarray(lidar2img, np.float32),
                  w_view=w_view, post_gamma=post_gamma, post_beta=post_beta,
                  logits=logits, patch_h=patch_h, patch_w=patch_w)
    plan, nc = _get_program(inputs["lidar2img"], patch_h, patch_w)
    in_maps = make_in_maps(inputs, plan)
    res = run_bass_kernel_spmd(nc, in_maps, core_ids=list(range(NCORE)),
                               trace=_trace)
    out = assemble_output(res.results, plan)
    kernel.last_result = res
    return out

